# revision 13
# baseline (speedup 1.0000x reference)
"""Multi-head attention (cosine-similarity scores, q=k=v) on 8 trn2 cores.

Reference computation (per head h, batch b):
    h_bh = sin_b @ Wx_h + bx_h                       # [S, F]
    C    = (h_bh h_bh^T) / (|h_s||h_t|)              # cosine scores, symmetric
    P    = softmax(C, axis=-1)                       # no max-shift needed: |C|<=1
    out_bh = P @ h_bh                                # [S, F]
    out_b  = concat_h(out_bh) @ Wp + bp              # [S, D]

Sharding: tensor-parallel over heads. Each core owns HPC=2 heads, computes the
partial output projection for its heads over the full batch, and the host sums
the 8 partials (+bp).

Per-core kernel layout trick: all score/value matmuls run in the "column"
orientation [t-partition, s-free]. Because C is symmetric, exp(C)[t,s] stored
column-wise is exactly the E[s,t] operand needed for Y^T = h^T E, so no
on-chip transpose of the 2048x2048 score matrix is ever needed. The softmax
denominator comes for free from a ones-column appended to the value stationary
operand (out partition 64 of the Y psum accumulates sum_t E[t,s]).

All matmuls use float32r (TF32) operands at full PE rate.
"""
import numpy as np

import concourse.bacc as bacc
import concourse.tile as tile
import concourse.mybir as mybir
from concourse import masks
from concourse.bass_utils import run_bass_kernel_spmd

B, S, D, H, F = 4, 2048, 1024, 16, 64
NCORES = 8
HPC = H // NCORES          # 2 heads per core
FL = HPC * F               # 128 local feature columns
SCH = 512                  # s-chunk (matmul moving dim)
NCH = S // SCH             # 4
KT = D // 128              # 8 contraction tiles for the input projection
NT0 = S // 128             # 16 t-blocks

DEBUG_DUMPS = False

FP = mybir.dt.float32
FPR = mybir.dt.float32r
AF = mybir.ActivationFunctionType


def _build_nc():
    nc = bacc.Bacc("TRN2", target_bir_lowering=False, debug=False)

    sinT = nc.dram_tensor("sinT", [B, D, S], FPR, kind="ExternalInput")
    wxl = nc.dram_tensor("wxl", [128, KT * FL], FPR, kind="ExternalInput")
    bxl = nc.dram_tensor("bxl", [FL, 1], FP, kind="ExternalInput")
    wpl = nc.dram_tensor("wpl", [FL, D], FPR, kind="ExternalInput")
    outp = nc.dram_tensor("outp", [B, S, D], FP, kind="ExternalOutput")
    dbg = {}
    if DEBUG_DUMPS:
        dbg["hT"] = nc.dram_tensor("dbg_hT", [128, S], FP, kind="ExternalOutput")
        dbg["norm0"] = nc.dram_tensor("dbg_norm0", [1, S], FP, kind="ExternalOutput")
        dbg["rnb"] = nc.dram_tensor("dbg_rnb", [128, S], FP, kind="ExternalOutput")
        dbg["hTn"] = nc.dram_tensor("dbg_hTn", [128, S], FP, kind="ExternalOutput")
        dbg["aug0"] = nc.dram_tensor("dbg_aug0", [128, NT0 * (F + 1)], FP, kind="ExternalOutput")
        dbg["outT"] = nc.dram_tensor("dbg_outT", [128, S], FP, kind="ExternalOutput")
        dbg["E00"] = nc.dram_tensor("dbg_E00", [128, SCH], FP, kind="ExternalOutput")
        dbg["psy00"] = nc.dram_tensor("dbg_psy00", [F + 1, SCH], FP, kind="ExternalOutput")

    with tile.TileContext(nc) as tc:
        with (
            tc.tile_pool(name="const", bufs=1) as constp,
            tc.tile_pool(name="wpool", bufs=1) as wpool,
            tc.tile_pool(name="sin", bufs=12) as sinp,
            tc.tile_pool(name="pa", bufs=1) as pa,
            tc.tile_pool(name="pb", bufs=2) as pb,
            tc.tile_pool(name="epool", bufs=3) as epool,
            tc.tile_pool(name="tail", bufs=2) as tailp,
            tc.tile_pool(name="opool", bufs=4) as opool,
            # 8 PSUM banks total: mm pool 4 x [128,512] (1 bank each) shared
            # by projection/scores/output-proj, small pool 2 x 1 bank
            # (transposes + norm reductions), y pool 2 x 1 bank (accumulators)
            tc.tile_pool(name="ps_mm", bufs=4, space="PSUM") as ps_mm,
            tc.tile_pool(name="ps_sm", bufs=2, space="PSUM") as ps_sm,
            tc.tile_pool(name="ps_y", bufs=2, space="PSUM") as ps_y,
        ):
            # ---- constants / weights ----
            ident = constp.tile([128, 128], FP, tag="ident")
            masks.make_identity(nc, ident[:])

            # ones2: column h is 1.0 on the 64 partitions of head h (for the
            # norm^2 partition-reduction matmul)
            ones2_f = constp.tile([128, 2], FP, tag="ones2f")
            nc.vector.memset(ones2_f[:], 0.0)
            nc.vector.memset(ones2_f[0:64, 0:1], 1.0)
            nc.vector.memset(ones2_f[64:128, 1:2], 1.0)
            ones2 = constp.tile([128, 2], FPR, tag="ones2")
            nc.vector.tensor_copy(ones2[:], ones2_f[:])

            ones16_f = constp.tile([128, NT0], FP, tag="ones16f")
            nc.vector.memset(ones16_f[:], 1.0)

            wx_t = wpool.tile([128, KT * FL], FPR, tag="wx")
            nc.sync.dma_start(wx_t[:], wxl.ap())
            bx_t = wpool.tile([FL, 1], FP, tag="bx")
            nc.sync.dma_start(bx_t[:], bxl.ap())
            wp_t = wpool.tile([FL, D], FPR, tag="wp")
            nc.sync.dma_start(wp_t[:], wpl.ap())

            for b in range(B):
                # ================= phase A: projection, norms, transposes ===
                # hT: [f_local(128-part), s] biased projection, fp32
                hT = pa.tile([128, S], FP, tag="hT")
                sqt = pa.tile([128, S], FPR, tag="sq")
                norms = [
                    pa.tile([1, S], FP, tag=f"norm{h}", name=f"norm{h}_{b}")
                    for h in range(HPC)
                ]
                for c in range(NCH):
                    cs = slice(c * SCH, (c + 1) * SCH)
                    pshT = ps_mm.tile([128, SCH], FP, tag="mm", name=f"pshT_{b}_{c}")
                    for k in range(KT):
                        sint = sinp.tile([128, SCH], FPR, tag="sin")
                        nc.sync.dma_start(
                            sint[:], sinT.ap()[b, k * 128:(k + 1) * 128, cs]
                        )
                        nc.tensor.matmul(
                            pshT[:], wx_t[:, k * FL:(k + 1) * FL], sint[:],
                            start=(k == 0), stop=(k == KT - 1),
                        )
                    nc.vector.tensor_scalar_add(hT[:, cs], pshT[:], bx_t[:])
                    # squares (fp32r) for the norm reduction
                    nc.scalar.activation(sqt[:, cs], hT[:, cs], AF.Square)
                    for h in range(HPC):
                        psn = ps_sm.tile([1, SCH], FP, tag="sm",
                                         name=f"psn_{b}_{c}_{h}")
                        nc.tensor.matmul(psn[:], ones2[:, h:h + 1], sqt[:, cs],
                                         start=True, stop=True)
                        nc.scalar.sqrt(norms[h][:, cs], psn[:])

                # partition_broadcast writes at the tile base partition only
                # (out partition offsets are silently dropped on HW), so
                # broadcast into base-0 tiles and copy into the upper half.
                rnb = pa.tile([128, S], FP, tag="rnb")
                rnb1 = pa.tile([F, S], FP, tag="rnb1")
                nc.vector.reciprocal(norms[0][:], norms[0][:])
                nc.gpsimd.partition_broadcast(rnb[0:F, :], norms[0][:])
                nc.vector.reciprocal(norms[1][:], norms[1][:])
                nc.gpsimd.partition_broadcast(rnb1[:], norms[1][:])
                nc.vector.tensor_copy(rnb[F:2 * F, :], rnb1[:])

                # normalized hT (fp32r) for the score matmuls
                hTn = pb.tile([128, S], FPR, tag="hTn")
                nc.vector.tensor_mul(hTn[:], hT[:], rnb[:])

                # aug[h]: per t-block [t(128-part), 64 h-cols + ones col]
                augs = [
                    pb.tile([128, NT0 * (F + 1)], FPR, tag=f"aug{h}",
                            name=f"aug{h}_{b}")
                    for h in range(HPC)
                ]
                for t0 in range(NT0):
                    pst = ps_sm.tile([128, 128], FP, tag="sm", name=f"pst_{b}_{t0}")
                    nc.tensor.transpose(
                        pst[:], hT[:, t0 * 128:(t0 + 1) * 128], ident[:]
                    )
                    for h in range(HPC):
                        nc.vector.tensor_copy(
                            augs[h][:, t0 * (F + 1):t0 * (F + 1) + F],
                            pst[:, h * F:(h + 1) * F],
                        )
                for h in range(HPC):
                    ones_col = augs[h][:].rearrange(
                        "p (i c) -> p i c", c=F + 1
                    )[:, :, F:F + 1]
                    nc.vector.tensor_copy(ones_col, ones16_f[:])

                if DEBUG_DUMPS and b == 0:
                    nc.sync.dma_start(dbg["hT"].ap(), hT[:])
                    nc.sync.dma_start(dbg["norm0"].ap(), norms[0][:])
                    nc.sync.dma_start(dbg["rnb"].ap(), rnb[:])
                    dbg_hTn_f = pa.tile([128, S], FP, tag="dbgf1")
                    nc.vector.tensor_copy(dbg_hTn_f[:], hTn[:])
                    nc.sync.dma_start(dbg["hTn"].ap(), dbg_hTn_f[:])
                    dbg_aug_f = pa.tile([128, NT0 * (F + 1)], FP, tag="dbgf2")
                    nc.vector.tensor_copy(dbg_aug_f[:], augs[0][:])
                    nc.sync.dma_start(dbg["aug0"].ap(), dbg_aug_f[:])

                # ================= phase B: attention =======================
                # outT: [f_local(128-part), s] = (P @ h)^T per head, fp32r
                outT = pb.tile([128, S], FPR, tag="outT")
                for h in range(HPC):
                    hr = slice(h * F, (h + 1) * F)
                    aug = augs[h]
                    for q in range(NCH):
                        qs = slice(q * SCH, (q + 1) * SCH)
                        psy = ps_y.tile([F + 1, SCH], FP, tag="psy", name=f"psy_{b}_{h}_{q}")
                        for t0 in range(NT0):
                            psc = ps_mm.tile([128, SCH], FP, tag="mm", name=f"psc_{b}_{h}_{q}_{t0}")
                            nc.tensor.matmul(
                                psc[:],
                                hTn[hr, t0 * 128:(t0 + 1) * 128],
                                hTn[hr, qs],
                                start=True, stop=True,
                            )
                            et = epool.tile([128, SCH], FPR, tag="E")
                            nc.scalar.activation(et[:], psc[:], AF.Exp)
                            if DEBUG_DUMPS and b == 0 and h == 0 and q == 0 and t0 == 0:
                                dbg_e_f = epool.tile([128, SCH], FP, tag="dbgf4")
                                nc.vector.tensor_copy(dbg_e_f[:], et[:])
                                nc.sync.dma_start(dbg["E00"].ap(), dbg_e_f[:])
                            nc.tensor.matmul(
                                psy[:],
                                aug[:, t0 * (F + 1):(t0 + 1) * (F + 1)],
                                et[:],
                                start=(t0 == 0), stop=(t0 == NT0 - 1),
                            )
                        if DEBUG_DUMPS and b == 0 and h == 0 and q == 0:
                            dbg_psy_f = tailp.tile([F + 1, SCH], FP, tag="dbgf3")
                            nc.vector.tensor_copy(dbg_psy_f[:], psy[:])
                            nc.sync.dma_start(dbg["psy00"].ap(), dbg_psy_f[:])
                        rd = tailp.tile([1, SCH], FP, tag="rd")
                        nc.vector.reciprocal(rd[:], psy[F:F + 1, :])
                        rdb = tailp.tile([F, SCH], FP, tag="rdb")
                        nc.gpsimd.partition_broadcast(rdb[:], rd[:])
                        nc.vector.tensor_mul(outT[hr, qs], psy[0:F, :], rdb[:])

                if DEBUG_DUMPS and b == 0:
                    dbg_outT_f = pa.tile([128, S], FP, tag="dbgf5")
                    nc.vector.tensor_copy(dbg_outT_f[:], outT[:])
                    nc.sync.dma_start(dbg["outT"].ap(), dbg_outT_f[:])

                # ================= phase C: output projection ===============
                for sb in range(S // 128):
                    ss = slice(sb * 128, (sb + 1) * 128)
                    ot = opool.tile([128, D], FP, tag="osb")
                    for n in range(D // 512):
                        psp = ps_mm.tile([128, 512], FP, tag="mm",
                                         name=f"psp_{b}_{sb}_{n}")
                        nc.tensor.matmul(
                            psp[:],
                            outT[:, ss],
                            wp_t[:, n * 512:(n + 1) * 512],
                            start=True, stop=True,
                        )
                        nc.vector.tensor_copy(ot[:, n * 512:(n + 1) * 512],
                                              psp[:])
                    nc.sync.dma_start(outp.ap()[b, ss, :], ot[:])

    nc.compile()
    return nc


_NC_CACHE = []


def _get_nc():
    if not _NC_CACHE:
        _NC_CACHE.append(_build_nc())
    return _NC_CACHE[0]


def make_in_maps(sin, Wx, bx, Wp):
    """Host-side sharding: per-core input dicts."""
    sinT = np.ascontiguousarray(np.transpose(sin, (0, 2, 1)))  # [B, D, S]
    in_maps = []
    for c in range(NCORES):
        hs = slice(c * HPC, (c + 1) * HPC)
        # [D, FL] stacked head projections -> [128, KT*FL] k-tile-major
        wxl = np.concatenate([Wx[h] for h in range(c * HPC, (c + 1) * HPC)],
                             axis=1)
        wxl = np.ascontiguousarray(
            wxl.reshape(KT, 128, FL).transpose(1, 0, 2).reshape(128, KT * FL)
        )
        bxl = np.ascontiguousarray(bx[hs].reshape(FL, 1))
        wpl = np.ascontiguousarray(Wp[c * FL:(c + 1) * FL, :])
        in_maps.append({"sinT": sinT, "wxl": wxl, "bxl": bxl, "wpl": wpl})
    return in_maps


def benchmark(sin, Wx, bx, Wp, iters=10):
    """Timed loop of the compiled executable with device-resident inputs.

    Outputs are fed back as the donated output buffers, so each iteration is
    dispatch + device execution only (no host transfers).
    """
    import time as _time

    import jax
    from concourse import bass2jax as b2j
    from concourse import mybir as _mb

    nc = _get_nc()
    b2j.install_neuronx_cc_hook()
    in_maps = make_in_maps(
        np.asarray(sin, np.float32), np.asarray(Wx, np.float32),
        np.asarray(bx, np.float32), np.asarray(Wp, np.float32),
    )

    in_names, out_names, out_avals, zero_outs = [], [], [], []
    for alloc in nc.m.functions[0].allocations:
        if not isinstance(alloc, _mb.MemoryLocationSet):
            continue
        name = alloc.memorylocations[0].name
        if alloc.kind == "ExternalInput":
            if nc.partition_id_tensor is None or name != nc.partition_id_tensor.name:
                in_names.append(name)
        elif alloc.kind == "ExternalOutput":
            out_names.append(name)
            shape = tuple(alloc.tensor_shape)
            dtype = _mb.dt.np(alloc.dtype)
            out_avals.append(jax.core.ShapedArray(shape, dtype))
            zero_outs.append(np.zeros(shape, dtype))
    n_params = len(in_names)
    n_outs = len(out_avals)
    all_names = in_names + out_names
    donate = tuple(range(n_params, n_params + n_outs))

    pid_name = nc.partition_id_tensor.name if nc.partition_id_tensor else None
    body_names = all_names + ([pid_name] if pid_name else [])

    def _body(*args):
        operands = list(args)
        if pid_name:
            operands.append(b2j.partition_id_tensor())
        outs = b2j._bass_exec_p.bind(
            *operands,
            out_avals=tuple(out_avals),
            in_names=tuple(body_names),
            out_names=tuple(out_names),
            lowering_input_output_aliases=(),
            sim_require_finite=True,
            sim_require_nnan=True,
            nc=nc,
        )
        return tuple(outs)

    devices = jax.devices()[:NCORES]
    mesh = b2j.Mesh(np.asarray(devices), ("core",))
    in_specs = (b2j.PartitionSpec("core"),) * (n_params + n_outs)
    out_specs = (b2j.PartitionSpec("core"),) * n_outs
    sharded = jax.jit(
        b2j.shard_map(_body, mesh=mesh, in_specs=in_specs,
                      out_specs=out_specs, check_rep=False),
        donate_argnums=donate, keep_unused=True,
    )
    sharding = jax.sharding.NamedSharding(mesh, b2j.PartitionSpec("core"))
    concat_in = [
        jax.device_put(
            np.concatenate([np.asarray(in_maps[c][nm]) for c in range(NCORES)],
                           axis=0),
            sharding,
        )
        for nm in in_names
    ]
    outs = [
        jax.device_put(np.zeros((NCORES * z.shape[0], *z.shape[1:]), z.dtype),
                       sharding)
        for z in zero_outs
    ]
    jax.block_until_ready(concat_in)

    times = []
    for _ in range(iters):
        t0 = _time.perf_counter()
        outs = sharded(*concat_in, *outs)
        jax.block_until_ready(outs)
        times.append((_time.perf_counter() - t0) * 1e9)
    return times


def kernel(sin, Wx, bx, Wp, bp, _trace=False):
    sin = np.asarray(sin, dtype=np.float32)
    Wx = np.asarray(Wx, dtype=np.float32)
    bx = np.asarray(bx, dtype=np.float32)
    Wp = np.asarray(Wp, dtype=np.float32)
    bp = np.asarray(bp, dtype=np.float32)

    nc = _get_nc()
    in_maps = make_in_maps(sin, Wx, bx, Wp)
    res = run_bass_kernel_spmd(nc, in_maps, list(range(NCORES)), trace=_trace)
    out = np.sum(np.stack([r["outp"] for r in res.results]), axis=0) + bp
    if _trace:
        kernel.last_results = res
    return out.astype(np.float32)
